# revision 29
# baseline (speedup 1.0000x reference)
"""Trainium2 Bass kernel: exact 3D Euclidean distance transform of a binary
(16, 512, 512) float32 volume — distance from every nonzero voxel to the
nearest zero voxel over ALL three axes (batch participates in the metric),
matching scipy.ndimage.distance_transform_edt on the full array.

Algorithm (separable min-plus with squared-parabola kernels; order-free):
  pass W: exact 1D nearest-zero distance along W via fwd/bwd saturating
          scans (tensor_tensor_scan, values clamp at CLAMP=32 == +inf),
          then square (PE transpose + ACT Square).
  pass B: banded parabola min-plus along B (|delta| <= R).
  pass H: banded parabola min-plus along H (|delta| <= R).
Banding is exact whenever the true max distance <= R; this is verified on
the host after the run (max over the output), with an exact host fallback
otherwise.  All intermediates are small integers (<= CLAMP^2 + R^2), exact
in fp16, which unlocks the DVE 2x perf mode.

Sharding: kernel 1 is data-parallel over H (8 slabs of 64 rows; the W-scan
and the B-pass need full W and full B, which each slab has).  The squared
intermediate is stored w-major; the host reshards it W-wise (numpy slicing)
and kernel 2 (data-parallel over W, 8 slabs of 64 cols) runs the H-pass,
which needs full H, plus the final sqrt.

Hardware quirk driving the structure: several instruction encodings
(DMA DIRECT2D, the S2S2D2 tensor-scalar-ptr family used by the scans and
scalar_tensor_tensor) accept only ONE semaphore wait.  Tile emits a wait
only when an engine's vector clock is behind, so the kernel is arranged so
every such instruction has at most one not-yet-observed cross-engine
dependency, and each kernel issues at most 8 HWDGE DMAs (no lane reuse).
"""
import numpy as np

B, H, W = 16, 512, 512
NCORES = 8
HS = H // NCORES
WS = W // NCORES
P = 128
CLAMP = 32.0
R = 5

_BUILT = None
LAST_RESULTS = []   # BassKernelResults of the most recent kernel() call


def _k1_body(tc, d2t_d, xs_d):
    """Pass W scans + PE transpose/ACT square + banded pass B.

    xs_d:  [16, HS, 512] f32 dram (ExternalInput)
    d2t_d: [512, 16, HS] f16 dram (ExternalOutput), squared distances

    Engine balance: both scans run on DVE (the bwd scan runs on the fwd
    result, which already equals min(fwd, d0), so its output is
    min(fwd, bwd) directly — no separate combine).  The banded pass-B adds
    are split between ACT (Copy+bias) and DVE (tensor_scalar); the mins run
    on DVE (tensor_tensor, 2x mode: every operand offset is a multiple of
    HS=64 fp16 elements, so alignment holds).
    """
    import concourse.mybir as mybir

    nc = tc.nc
    f16 = mybir.dt.float16
    f32 = mybir.dt.float32
    Alu = mybir.AluOpType
    Act = mybir.ActivationFunctionType
    N_T = (B * HS) // P       # 8 scan tiles
    N_J = W // P              # 4 w-groups
    C = B * HS                # 1024 free elements per w after transpose

    from concourse.masks import make_identity

    with tc.tile_pool(name="const", bufs=1) as cpool, \
         tc.tile_pool(name="big", bufs=1) as bpool, \
         tc.tile_pool(name="tmp", bufs=3) as tpool, \
         tc.tile_pool(name="psum", bufs=7, space="PSUM") as ppool, \
         tc.tile_pool(name="psumw", bufs=1, space="PSUM") as ppoolw:

        ident = cpool.tile([P, P], f16)
        make_identity(nc, ident[:])
        ones = cpool.tile([P, W], f16)
        nc.vector.memset(ones[:], 1.0)
        # dummy transpose so PE observes the gpsimd-built identity before the
        # real transposes (keeps every matmul at <= 1 semaphore wait)
        psw = ppoolw.tile([P, P], f16)
        nc.tensor.transpose(psw[:], ident[:], ident[:])

        XH = bpool.tile([P, N_T * W], f16)      # x cast to f16 by the DMA
        AALL = bpool.tile([P, N_T * W], f16)    # d0 = (x != 0) * CLAMP
        FALL = bpool.tile([P, N_T * W], f16)    # fwd scan
        DALL = bpool.tile([P, N_T * W], f16)    # bwd scan of fwd = 1D dist

        qs = N_T * W // 4
        for m in range(4):
            nc.gpsimd.dma_start(
                XH[:, qs * m: qs * (m + 1)].rearrange(
                    "p (g w) -> p g w", g=2),
                xs_d[4 * m: 4 * (m + 1)].rearrange(
                    "(g bb) h w -> (bb h) g w", g=2, bb=2))
        for q in range(4):
            qs = N_T * W // 4
            nc.vector.tensor_scalar(
                AALL[:, qs * q: qs * (q + 1)],
                XH[:, qs * q: qs * (q + 1)],
                0.0, CLAMP, Alu.not_equal, Alu.mult)

        for t in range(N_T):
            fa = FALL[:, W * t: W * (t + 1)]
            nc.vector.tensor_tensor_scan(
                fa, ones[:], AALL[:, W * t: W * (t + 1)], CLAMP,
                Alu.add, Alu.min)
            nc.vector.tensor_tensor_scan(
                DALL[:, W * t: W * (t + 1)][:, ::-1], ones[:], fa[:, ::-1],
                CLAMP, Alu.add, Alu.min)

        SQ = bpool.tile([P, N_J * C], f16)      # w lines x (j, b, h)
        for j in range(N_J):
            ps = ppool.tile([P, C], f16, tag="ps")
            for t in range(N_T):
                nc.tensor.transpose(
                    ps[:, P * t:P * (t + 1)],
                    DALL[:, W * t + P * j: W * t + P * (j + 1)], ident[:])
            nc.scalar.activation(SQ[:, C * j:C * (j + 1)], ps[:], Act.Square)

        # banded pass B: per shift, an add (ACT or DVE tensor_scalar) then a
        # DVE tensor_tensor min.  ACC is initialized by the first (s=1) min
        # for b<15 plus a small ACT copy for the b=15 strip.
        ACC = bpool.tile([P, N_J * C], f16)
        sq4 = SQ[:].rearrange("p (j b h) -> p j b h", j=N_J, b=B)
        ac4 = ACC[:].rearrange("p (j b h) -> p j b h", j=N_J, b=B)
        nc.scalar.activation(ac4[:, :, B - 1:B, :], sq4[:, :, B - 1:B, :],
                             Act.Copy, bias=0.0)
        first = True
        for s in range(1, R + 1):
            bc = B - s
            for sgn in (1, -1):
                if sgn > 0:
                    srcv = sq4[:, :, s:s + bc, :]
                    outv = ac4[:, :, 0:bc, :]
                else:
                    srcv = sq4[:, :, 0:bc, :]
                    outv = ac4[:, :, s:B, :]
                tmp = tpool.tile([P, N_J * C], f16, tag="tmp")
                tmpv = tmp[:].rearrange(
                    "p (j b h) -> p j b h", j=N_J, b=B)[:, :, 0:bc, :]
                if s >= 2:
                    nc.scalar.activation(tmpv, srcv, Act.Copy,
                                         bias=float(s * s))
                else:
                    nc.vector.tensor_scalar(tmpv, srcv, float(s * s), None,
                                            Alu.add)
                if first:
                    # acc := min(sq (delta=0), tmp) initializes b < 15
                    nc.vector.tensor_tensor(outv, tmpv, sq4[:, :, 0:bc, :],
                                            Alu.min)
                    first = False
                elif s == R and sgn == -1:
                    # split the final fold by j-halves so the two output DMAs
                    # can start as soon as their half is done
                    jh = N_J // 2
                    nc.vector.tensor_tensor(outv[:, 0:jh], tmpv[:, 0:jh],
                                            outv[:, 0:jh], Alu.min)
                    nc.vector.tensor_tensor(outv[:, jh:N_J], tmpv[:, jh:N_J],
                                            outv[:, jh:N_J], Alu.min)
                else:
                    nc.vector.tensor_tensor(outv, tmpv, outv, Alu.min)

        d2tv = d2t_d.rearrange("(j p) b h -> p j (b h)", p=P)
        accv = ACC[:].rearrange("p (j c) -> p j c", j=N_J)
        jh = N_J // 2
        nc.sync.dma_start(d2tv[:, 0:jh], accv[:, 0:jh])
        nc.scalar.dma_start(d2tv[:, jh:N_J], accv[:, jh:N_J])


HB = 8                    # input halo rows per side (>= R)
HE = HS + 2 * HB          # 80 extended rows per core
N_T3 = (B * HE) // P      # 10 scan tiles


def _k3_body(tc, out_d, xs_d):
    """Fused single-launch EDT: W-scans + transpose/square + banded H-pass +
    banded B-pass + sqrt.  Needs only an input halo of HB >= R rows (host
    pads with foreground), so no cross-core communication at all.

    xs_d:  [16, HE, 512] f32 dram (ExternalInput, host-padded h-slab)
    out_d: [512, 16, HS] f32 dram (ExternalOutput), distances, w-major
    """
    import concourse.mybir as mybir

    nc = tc.nc
    f16 = mybir.dt.float16
    f32 = mybir.dt.float32
    Alu = mybir.AluOpType
    Act = mybir.ActivationFunctionType
    N_J = W // P              # 4 w-groups
    CE = B * HE               # 1280 lines, also transposed free size per j
    C = B * HS                # 1024 interior (b,h) elements per w

    from concourse.masks import make_identity

    with tc.tile_pool(name="const", bufs=1) as cpool, \
         tc.tile_pool(name="big", bufs=1) as bpool, \
         tc.tile_pool(name="tmp", bufs=3) as tpool, \
         tc.tile_pool(name="psum", bufs=3, space="PSUM") as ppool, \
         tc.tile_pool(name="psumw", bufs=1, space="PSUM") as ppoolw:

        ident = cpool.tile([P, P], f16)
        make_identity(nc, ident[:])
        ones = cpool.tile([P, W], f16)
        nc.vector.memset(ones[:], 1.0)
        psw = ppoolw.tile([P, P], f16)
        nc.tensor.transpose(psw[:], ident[:], ident[:])

        XH = bpool.tile([P, N_T3 * W], f16)
        AALL = bpool.tile([P, N_T3 * W], f16)
        FALL = bpool.tile([P, N_T3 * W], f16)
        DALL = bpool.tile([P, N_T3 * W], f16)

        # the padded slab is a flat [1280, 512] row matrix; 5 casting DMAs
        xflat = xs_d.rearrange("b h w -> (b h) w")
        for m in range(5):
            nc.gpsimd.dma_start(
                XH[:, 2 * W * m: 2 * W * (m + 1)].rearrange(
                    "p (g w) -> p g w", g=2),
                xflat[256 * m: 256 * (m + 1)].rearrange(
                    "(g pp) w -> pp g w", g=2))
        for m in range(5):
            nc.vector.tensor_scalar(
                AALL[:, 2 * W * m: 2 * W * (m + 1)],
                XH[:, 2 * W * m: 2 * W * (m + 1)],
                0.0, CLAMP, Alu.not_equal, Alu.mult)

        for t in range(N_T3):
            fa = FALL[:, W * t: W * (t + 1)]
            nc.vector.tensor_tensor_scan(
                fa, ones[:], AALL[:, W * t: W * (t + 1)], CLAMP,
                Alu.add, Alu.min)
            nc.vector.tensor_tensor_scan(
                DALL[:, W * t: W * (t + 1)][:, ::-1], ones[:], fa[:, ::-1],
                CLAMP, Alu.add, Alu.min)

        SQ = bpool.tile([P, N_J * CE], f16)     # w lines x (j, b, h80)
        for j in range(N_J):
            # two PSUM tiles per j: 640 fp16 = 1280B stays inside one 2KB
            # PSUM bank (a straddling AP faults the exec unit)
            for hf in range(2):
                ps = ppool.tile([P, CE // 2], f16, tag="ps")
                for tt_ in range(N_T3 // 2):
                    t = hf * (N_T3 // 2) + tt_
                    nc.tensor.transpose(
                        ps[:, P * tt_:P * (tt_ + 1)],
                        DALL[:, W * t + P * j: W * t + P * (j + 1)], ident[:])
                nc.scalar.activation(
                    SQ[:, CE * j + (CE // 2) * hf:
                       CE * j + (CE // 2) * (hf + 1)], ps[:], Act.Square)

        # SQB[c] = SQ[c+1]: 4B-aligned source for odd H-shifts
        SQB = bpool.tile([P, N_J * CE], f16)
        nc.vector.tensor_copy(SQB[:, 0:N_J * CE - 1], SQ[:, 1:N_J * CE])

        # banded pass H: pair = min(left, right), tmp = pair + s^2,
        # accH = min(accH, tmp).  Output interior h in [HB, HB+HS).
        ACH = bpool.tile([P, N_J * C], f16)
        sq5 = SQ[:].rearrange("p (j b h) -> p j b h", j=N_J, b=B)
        sqb5 = SQB[:].rearrange("p (j b h) -> p j b h", j=N_J, b=B)
        ah4 = ACH[:].rearrange("p (j b h) -> p j b h", j=N_J, b=B)
        center = sq5[:, :, :, HB:HB + HS]
        order = sorted(range(1, R + 1), key=lambda s: (s % 2, -s))
        for si, s in enumerate(order):
            if s % 2 == 0:
                lo = sq5[:, :, :, HB - s:HB - s + HS]
                hi = sq5[:, :, :, HB + s:HB + s + HS]
            else:
                lo = sqb5[:, :, :, HB - s - 1:HB - s - 1 + HS]
                hi = sqb5[:, :, :, HB + s - 1:HB + s - 1 + HS]
            pair = tpool.tile([P, N_J * C], f16, tag="pair")
            pair4 = pair[:].rearrange("p (j b h) -> p j b h", j=N_J, b=B)
            if si == 0:
                jh = N_J // 2
                nc.vector.tensor_tensor(pair4[:, 0:jh], lo[:, 0:jh],
                                        hi[:, 0:jh], Alu.min)
                nc.vector.tensor_tensor(pair4[:, jh:N_J], lo[:, jh:N_J],
                                        hi[:, jh:N_J], Alu.min)
            else:
                nc.vector.tensor_tensor(pair4, lo, hi, Alu.min)
            tmp = tpool.tile([P, N_J * C], f16, tag="tmp")
            tmp4 = tmp[:].rearrange("p (j b h) -> p j b h", j=N_J, b=B)
            if s == 2:
                nc.vector.tensor_scalar(tmp4, pair4, float(s * s), None,
                                        Alu.add)
            else:
                nc.scalar.activation(tmp4, pair4, Act.Copy, bias=float(s * s))
            if si == 0:
                nc.vector.tensor_tensor(ah4, tmp4, center, Alu.min)
            else:
                nc.vector.tensor_tensor(ah4, tmp4, ah4, Alu.min)

        # banded pass B on the compact (j, b, h64) layout
        ACC = bpool.tile([P, N_J * C], f16)
        ac4 = ACC[:].rearrange("p (j b h) -> p j b h", j=N_J, b=B)
        nc.scalar.activation(ac4[:, :, B - 1:B, :], ah4[:, :, B - 1:B, :],
                             Act.Copy, bias=0.0)
        first = True
        for s in range(1, R + 1):
            bc = B - s
            for sgn in (1, -1):
                if sgn > 0:
                    srcv = ah4[:, :, s:s + bc, :]
                    outv = ac4[:, :, 0:bc, :]
                else:
                    srcv = ah4[:, :, 0:bc, :]
                    outv = ac4[:, :, s:B, :]
                tmp = tpool.tile([P, N_J * C], f16, tag="tmpb")
                tmpv = tmp[:].rearrange(
                    "p (j b h) -> p j b h", j=N_J, b=B)[:, :, 0:bc, :]
                if s >= 2:
                    nc.scalar.activation(tmpv, srcv, Act.Copy,
                                         bias=float(s * s))
                else:
                    nc.vector.tensor_scalar(tmpv, srcv, float(s * s), None,
                                            Alu.add)
                if first:
                    nc.vector.tensor_tensor(outv, tmpv, ah4[:, :, 0:bc, :],
                                            Alu.min)
                    first = False
                elif s == R and sgn == -1:
                    for j in range(N_J):
                        nc.vector.tensor_tensor(
                            outv[:, j:j + 1], tmpv[:, j:j + 1],
                            outv[:, j:j + 1], Alu.min)
                else:
                    nc.vector.tensor_tensor(outv, tmpv, outv, Alu.min)

        OUT = bpool.tile([P, N_J * C], f32)
        outd = out_d.rearrange("(j p) b h -> p j (b h)", p=P)
        outs = OUT[:].rearrange("p (j c) -> p j c", j=N_J)
        accs = ACC[:].rearrange("p (j c) -> p j c", j=N_J)
        for j in range(N_J):
            nc.scalar.activation(outs[:, j:j + 1], accs[:, j:j + 1], Act.Sqrt)
            eng = nc.sync if j % 2 == 0 else nc.scalar
            eng.dma_start(outd[:, j:j + 1], outs[:, j:j + 1])


def _k2_body(tc, out_d, d2s_d):
    """Banded pass H + sqrt.

    d2s_d: [WS, 16, 512] f16 dram (ExternalInput)
    out_d: [WS, 16, 512] f32 dram (ExternalOutput), distances

    Row-padded layout (PADH pad columns, value 2047 == +inf) so every
    shifted operand is full-width; odd shifts read SRCB (CALL displaced by
    one column) so all hot operands stay 4B-aligned.  Per |s| the two
    directions fold as pair = min(left, right) (DVE), tmp = pair + s^2
    (ACT, or DVE tensor_scalar for one shift), acc = min(acc, tmp) (DVE).
    """
    import concourse.mybir as mybir

    nc = tc.nc
    f16 = mybir.dt.float16
    f32 = mybir.dt.float32
    Alu = mybir.AluOpType
    Act = mybir.ActivationFunctionType
    N_G = (WS * B) // P       # 8 groups of 128 (w,b) lines
    PADH = 8
    HP = H + 2 * PADH         # 528 padded row width

    with tc.tile_pool(name="big", bufs=1) as bpool, \
         tc.tile_pool(name="tmp", bufs=3) as tpool:

        CALL = bpool.tile([P, N_G * HP], f16)
        SRCB = bpool.tile([P, N_G * HP], f16)
        ACC = bpool.tile([P, N_G * HP], f16)
        # pad strips (+inf) for CALL rows [0:PADH) and [PADH+H:HP),
        # for SRCB rows [0:PADH-1) and [PADH-1+H:HP)
        cstrips = CALL[:].rearrange("p (g h) -> p g h", g=N_G)
        nc.gpsimd.memset(cstrips[:, :, 0:PADH], 2047.0)
        nc.gpsimd.memset(cstrips[:, :, PADH + H:HP], 2047.0)
        strips = SRCB[:].rearrange("p (g h) -> p g h", g=N_G)
        nc.gpsimd.memset(strips[:, :, 0:PADH - 1], 2047.0)
        nc.gpsimd.memset(strips[:, :, PADH - 1 + H:HP], 2047.0)

        call3 = CALL[:].rearrange("p (g h) -> p g h", g=N_G)
        srcb3 = SRCB[:].rearrange("p (g h) -> p g h", g=N_G)
        acc3 = ACC[:].rearrange("p (g h) -> p g h", g=N_G)
        accint = acc3[:, :, PADH:PADH + H]
        d2sv = d2s_d.rearrange("(g ww) b h -> (ww b) g (h)",
                               g=N_G, ww=WS // N_G)
        # three copies of the input land by DMA: source, 1-column-shifted
        # source (odd-shift alignment helper), and the accumulator init.
        nc.sync.dma_start(call3[:, :, PADH:PADH + H], d2sv)
        nc.scalar.dma_start(srcb3[:, :, PADH - 1:PADH - 1 + H], d2sv)
        nc.sync.dma_start(accint, d2sv)

        order = sorted(range(1, R + 1), key=lambda s: (s % 2, -s))
        assert order[-1] == 1
        for s in order:
            if s % 2 == 0:
                lo = call3[:, :, PADH - s:PADH - s + H]
                hi = call3[:, :, PADH + s:PADH + s + H]
            else:
                lo = srcb3[:, :, PADH - s - 1:PADH - s - 1 + H]
                hi = srcb3[:, :, PADH + s - 1:PADH + s - 1 + H]
            pair = tpool.tile([P, N_G * H], f16, tag="pair")
            pair3 = pair[:].rearrange("p (g h) -> p g h", g=N_G)
            nc.vector.tensor_tensor(pair3, lo, hi, Alu.min)
            tmp = tpool.tile([P, N_G * H], f16, tag="tmp")
            tmp3 = tmp[:].rearrange("p (g h) -> p g h", g=N_G)
            if s == 2:
                nc.vector.tensor_scalar(tmp3, pair3, float(s * s), None,
                                        Alu.add)
            else:
                nc.scalar.activation(tmp3, pair3, Act.Copy, bias=float(s * s))
            if s == 1:
                # split the chain tail so sqrt + store of the first half
                # overlap the second half's min
                hh = N_G // 2
                nc.vector.tensor_tensor(accint[:, 0:hh], tmp3[:, 0:hh],
                                        accint[:, 0:hh], Alu.min)
                nc.vector.tensor_tensor(accint[:, hh:N_G], tmp3[:, hh:N_G],
                                        accint[:, hh:N_G], Alu.min)
            else:
                nc.vector.tensor_tensor(accint, tmp3, accint, Alu.min)

        OUT = bpool.tile([P, N_G * H], f32)
        out3 = OUT[:].rearrange("p (g h) -> p g h", g=N_G)
        outv = out_d.rearrange("(g ww) b h -> (ww b) g (h)",
                               g=N_G, ww=WS // N_G)
        hh = N_G // 2
        for half in range(2):
            sl = slice(hh * half, hh * (half + 1))
            nc.scalar.activation(out3[:, sl], accint[:, sl], Act.Sqrt)
            eng = nc.sync if half == 0 else nc.scalar
            eng.dma_start(outv[:, sl], out3[:, sl])


def _split_multi_waits(nc):
    """Walrus in this toolchain encodes at most ONE sync wait per hardware
    instruction.  Hoist extra waits onto same-engine NoOp carriers inserted
    immediately before the over-subscribed instruction (program order on the
    engine preserves the semantics exactly)."""
    import concourse.mybir as mybir

    n = 0
    for fn in nc.m.functions:
        for blk in fn.blocks:
            insts = blk.instructions
            out = []
            for inst in insts:
                si = inst.sync_info
                if si is not None and len(si.on_wait) > 1:
                    waits = list(si.on_wait)
                    for w in waits[:-1]:
                        nop = mybir.InstNoOp(
                            name=f"waitsplit-{n}", ins=[], outs=[])
                        n += 1
                        nop.engine = inst.engine
                        nop.sync_info = mybir.SyncInfo(
                            on_wait=[w], on_update=[])
                        out.append(nop)
                    inst.sync_info = mybir.SyncInfo(
                        on_wait=[waits[-1]], on_update=list(si.on_update))
                out.append(inst)
            blk.instructions = out
    return n


def _make_tc_class():
    """TileContext whose kernel-tail drain is split into one drain per proc.

    The stock tail emits a single sync-engine Drain waiting on every
    outstanding processor; this walrus build only encodes ONE sync wait per
    instruction, so the aggregated drain fails codegen.  Semantics are
    identical — the waits just land on consecutive Drain instructions.
    """
    import concourse.tile as tile
    from concourse.vector_clock import ScopedClock, VectorClock

    class SplitDrainTileContext(tile.TileContext):
        def _drain_and_barrier(self, tick_clock, wait_clock):
            gvc = tick_clock.global_clock
            for proc in range(len(gvc)):
                t = gvc[proc]
                if t <= 0:
                    continue
                d = self.nc.sync.drain()
                sv = VectorClock([0] * len(gvc))
                sv.require_at_least(proc, t)
                wait_clock.add_sem_waits(d.ins, ScopedClock({None: sv}))
            self.nc.all_engine_barrier()
            assert self.sems is not None
            popped = self.nc._tile_sem_poison_stack.pop()
            assert popped is self._sem_poison
            self.nc.clear_and_free_semaphores(
                list(self.sems.allocated().values()))
            self.nc.all_engine_barrier()

    return SplitDrainTileContext


def _build():
    """Build the fused Bass module (done once per process)."""
    import concourse.bass as bass
    import concourse.mybir as mybir

    f16 = mybir.dt.float16
    f32 = mybir.dt.float32
    TC = _make_tc_class()

    nc3 = bass.Bass("TRN2", debug=False, num_devices=NCORES)
    xs3_d = nc3.dram_tensor("xs3", [B, HE, W], f32,
                            kind="ExternalInput").ap()
    ot3_d = nc3.dram_tensor("ot3", [W, B, HS], f32,
                            kind="ExternalOutput").ap()
    with TC(nc3) as tc:
        _k3_body(tc, ot3_d, xs3_d)
    _split_multi_waits(nc3)
    return (nc3,)


def _build_two_launch():
    """Older two-launch pipeline (kept as reference/fallback)."""
    import concourse.bass as bass
    import concourse.mybir as mybir

    f16 = mybir.dt.float16
    f32 = mybir.dt.float32
    TC = _make_tc_class()

    nc1 = bass.Bass("TRN2", debug=False, num_devices=NCORES)
    xs_d = nc1.dram_tensor("xs", [B, HS, W], f32, kind="ExternalInput").ap()
    d2t_d = nc1.dram_tensor("d2t", [W, B, HS], f16, kind="ExternalOutput").ap()
    with TC(nc1) as tc:
        _k1_body(tc, d2t_d, xs_d)
    _split_multi_waits(nc1)

    nc2 = bass.Bass("TRN2", debug=False, num_devices=NCORES)
    d2s_d = nc2.dram_tensor("d2s", [WS, B, H], f16, kind="ExternalInput").ap()
    out_d = nc2.dram_tensor("ot", [WS, B, H], f32, kind="ExternalOutput").ap()
    with TC(nc2) as tc:
        _k2_body(tc, out_d, d2s_d)
    _split_multi_waits(nc2)

    return nc1, nc2


def _host_exact_edt(x):
    """Exact host fallback: banded numpy EDT with growing radius (f32)."""
    INF = np.float32(1e9)
    r = 2 * R
    while True:
        d0 = np.where(x != 0, INF, np.float32(0.0))
        fwd = np.empty_like(d0)
        st = np.full(d0.shape[:2], INF, np.float32)
        for w in range(W):
            st = np.minimum(st + 1.0, d0[:, :, w]); fwd[:, :, w] = st
        st = np.full(d0.shape[:2], INF, np.float32)
        bwd = np.empty_like(d0)
        for w in range(W - 1, -1, -1):
            st = np.minimum(st + 1.0, d0[:, :, w]); bwd[:, :, w] = st
        d2 = np.minimum(fwd, bwd) ** 2
        for axis in (0, 1):
            src = d2
            acc = src.copy()
            rr = min(r, x.shape[axis] - 1)
            for s in range(1, rr + 1):
                sl_lo = [slice(None)] * 3
                sl_hi = [slice(None)] * 3
                sl_lo[axis] = slice(0, x.shape[axis] - s)
                sl_hi[axis] = slice(s, None)
                np.minimum(acc[tuple(sl_lo)], src[tuple(sl_hi)] + s * s,
                           out=acc[tuple(sl_lo)])
                np.minimum(acc[tuple(sl_hi)], src[tuple(sl_lo)] + s * s,
                           out=acc[tuple(sl_hi)])
            d2 = acc
        out = np.sqrt(d2)
        # exact when every per-axis offset fits in the band; r >= max dim
        # means the bands are complete regardless of the value of out
        if out.max() <= r or r >= max(x.shape):
            return out.astype(np.float32)
        r *= 2


def kernel(x):
    global _BUILT
    x = np.asarray(x)
    assert x.shape == (B, H, W)
    if x.dtype != np.float32:
        x = x.astype(np.float32)

    from concourse.bass_utils import run_bass_kernel_spmd

    if _BUILT is None:
        _BUILT = _build()
    (nc3,) = _BUILT
    LAST_RESULTS.clear()

    core_ids = list(range(NCORES))
    # pad H with foreground (nonzero) halo rows; each core gets its slab
    # plus HB halo rows each side, so all three passes are core-local.
    xp = np.pad(x, ((0, 0), (HB, HB), (0, 0)), constant_values=1.0)
    in3 = [{"xs3": np.ascontiguousarray(xp[:, k * HS:k * HS + HE, :])}
           for k in range(NCORES)]
    r3 = run_bass_kernel_spmd(nc3, in3, core_ids)
    LAST_RESULTS.append(r3)
    outt = np.concatenate([r3.results[k]["ot3"] for k in range(NCORES)],
                          axis=2)

    out = outt.transpose(1, 2, 0)          # (w,b,h) -> (b,h,w)

    # Banding is exact iff the true max distance < R + 1 (per-axis integer
    # offsets of the optimal zero are bounded by floor of the distance, and
    # the banded result upper-bounds the true one).
    if float(np.max(out)) >= R + 1:
        out = _host_exact_edt(x)

    nan_mask = np.isnan(x)
    if nan_mask.any():
        out = np.where(nan_mask, np.float32(np.nan), out)
    return out


# revision 34
# speedup vs baseline: 1.0134x; 1.0134x over previous
"""Trainium2 Bass kernel: exact 3D Euclidean distance transform of a binary
(16, 512, 512) float32 volume — distance from every nonzero voxel to the
nearest zero voxel over ALL three axes (batch participates in the metric),
matching scipy.ndimage.distance_transform_edt on the full array.

Algorithm (separable min-plus with squared-parabola kernels; order-free):
  pass W: exact 1D nearest-zero distance along W via fwd/bwd saturating
          scans (tensor_tensor_scan, values clamp at CLAMP=32 == +inf),
          then square (PE transpose + ACT Square).
  pass B: banded parabola min-plus along B (|delta| <= R).
  pass H: banded parabola min-plus along H (|delta| <= R).
Banding is exact whenever the true max distance <= R; this is verified on
the host after the run (max over the output), with an exact host fallback
otherwise.  All intermediates are small integers (<= CLAMP^2 + R^2), exact
in fp16, which unlocks the DVE 2x perf mode.

Sharding: kernel 1 is data-parallel over H (8 slabs of 64 rows; the W-scan
and the B-pass need full W and full B, which each slab has).  The squared
intermediate is stored w-major; the host reshards it W-wise (numpy slicing)
and kernel 2 (data-parallel over W, 8 slabs of 64 cols) runs the H-pass,
which needs full H, plus the final sqrt.

Hardware quirk driving the structure: several instruction encodings
(DMA DIRECT2D, the S2S2D2 tensor-scalar-ptr family used by the scans and
scalar_tensor_tensor) accept only ONE semaphore wait.  Tile emits a wait
only when an engine's vector clock is behind, so the kernel is arranged so
every such instruction has at most one not-yet-observed cross-engine
dependency, and each kernel issues at most 8 HWDGE DMAs (no lane reuse).
"""
import numpy as np

B, H, W = 16, 512, 512
NCORES = 8
HS = H // NCORES
WS = W // NCORES
P = 128
CLAMP = 32.0
R = 5

_BUILT = None
LAST_RESULTS = []   # BassKernelResults of the most recent kernel() call


def _k1_body(tc, d2t_d, xs_d):
    """Pass W scans + PE transpose/ACT square + banded pass B.

    xs_d:  [16, HS, 512] f32 dram (ExternalInput)
    d2t_d: [512, 16, HS] f16 dram (ExternalOutput), squared distances

    Engine balance: both scans run on DVE (the bwd scan runs on the fwd
    result, which already equals min(fwd, d0), so its output is
    min(fwd, bwd) directly — no separate combine).  The banded pass-B adds
    are split between ACT (Copy+bias) and DVE (tensor_scalar); the mins run
    on DVE (tensor_tensor, 2x mode: every operand offset is a multiple of
    HS=64 fp16 elements, so alignment holds).
    """
    import concourse.mybir as mybir

    nc = tc.nc
    f16 = mybir.dt.float16
    f32 = mybir.dt.float32
    Alu = mybir.AluOpType
    Act = mybir.ActivationFunctionType
    N_T = (B * HS) // P       # 8 scan tiles
    N_J = W // P              # 4 w-groups
    C = B * HS                # 1024 free elements per w after transpose

    from concourse.masks import make_identity

    with tc.tile_pool(name="const", bufs=1) as cpool, \
         tc.tile_pool(name="big", bufs=1) as bpool, \
         tc.tile_pool(name="tmp", bufs=3) as tpool, \
         tc.tile_pool(name="psum", bufs=7, space="PSUM") as ppool, \
         tc.tile_pool(name="psumw", bufs=1, space="PSUM") as ppoolw:

        ident = cpool.tile([P, P], f16)
        make_identity(nc, ident[:])
        ones = cpool.tile([P, W], f16)
        nc.vector.memset(ones[:], 1.0)
        # dummy transpose so PE observes the gpsimd-built identity before the
        # real transposes (keeps every matmul at <= 1 semaphore wait)
        psw = ppoolw.tile([P, P], f16)
        nc.tensor.transpose(psw[:], ident[:], ident[:])

        XH = bpool.tile([P, N_T * W], f16)      # x cast to f16 by the DMA
        AALL = bpool.tile([P, N_T * W], f16)    # d0 = (x != 0) * CLAMP
        FALL = bpool.tile([P, N_T * W], f16)    # fwd scan
        DALL = bpool.tile([P, N_T * W], f16)    # bwd scan of fwd = 1D dist

        qs = N_T * W // 4
        for m in range(4):
            nc.gpsimd.dma_start(
                XH[:, qs * m: qs * (m + 1)].rearrange(
                    "p (g w) -> p g w", g=2),
                xs_d[4 * m: 4 * (m + 1)].rearrange(
                    "(g bb) h w -> (bb h) g w", g=2, bb=2))
        for q in range(4):
            qs = N_T * W // 4
            nc.vector.tensor_scalar(
                AALL[:, qs * q: qs * (q + 1)],
                XH[:, qs * q: qs * (q + 1)],
                0.0, CLAMP, Alu.not_equal, Alu.mult)

        for t in range(N_T):
            fa = FALL[:, W * t: W * (t + 1)]
            nc.vector.tensor_tensor_scan(
                fa, ones[:], AALL[:, W * t: W * (t + 1)], CLAMP,
                Alu.add, Alu.min)
            nc.vector.tensor_tensor_scan(
                DALL[:, W * t: W * (t + 1)][:, ::-1], ones[:], fa[:, ::-1],
                CLAMP, Alu.add, Alu.min)

        SQ = bpool.tile([P, N_J * C], f16)      # w lines x (j, b, h)
        for j in range(N_J):
            ps = ppool.tile([P, C], f16, tag="ps")
            for t in range(N_T):
                nc.tensor.transpose(
                    ps[:, P * t:P * (t + 1)],
                    DALL[:, W * t + P * j: W * t + P * (j + 1)], ident[:])
            nc.scalar.activation(SQ[:, C * j:C * (j + 1)], ps[:], Act.Square)

        # banded pass B: per shift, an add (ACT or DVE tensor_scalar) then a
        # DVE tensor_tensor min.  ACC is initialized by the first (s=1) min
        # for b<15 plus a small ACT copy for the b=15 strip.
        ACC = bpool.tile([P, N_J * C], f16)
        sq4 = SQ[:].rearrange("p (j b h) -> p j b h", j=N_J, b=B)
        ac4 = ACC[:].rearrange("p (j b h) -> p j b h", j=N_J, b=B)
        nc.scalar.activation(ac4[:, :, B - 1:B, :], sq4[:, :, B - 1:B, :],
                             Act.Copy, bias=0.0)
        first = True
        for s in range(1, R + 1):
            bc = B - s
            for sgn in (1, -1):
                if sgn > 0:
                    srcv = sq4[:, :, s:s + bc, :]
                    outv = ac4[:, :, 0:bc, :]
                else:
                    srcv = sq4[:, :, 0:bc, :]
                    outv = ac4[:, :, s:B, :]
                tmp = tpool.tile([P, N_J * C], f16, tag="tmp")
                tmpv = tmp[:].rearrange(
                    "p (j b h) -> p j b h", j=N_J, b=B)[:, :, 0:bc, :]
                if s >= 2:
                    nc.scalar.activation(tmpv, srcv, Act.Copy,
                                         bias=float(s * s))
                else:
                    nc.vector.tensor_scalar(tmpv, srcv, float(s * s), None,
                                            Alu.add)
                if first:
                    # acc := min(sq (delta=0), tmp) initializes b < 15
                    nc.vector.tensor_tensor(outv, tmpv, sq4[:, :, 0:bc, :],
                                            Alu.min)
                    first = False
                elif s == R and sgn == -1:
                    # split the final fold by j-halves so the two output DMAs
                    # can start as soon as their half is done
                    jh = N_J // 2
                    nc.vector.tensor_tensor(outv[:, 0:jh], tmpv[:, 0:jh],
                                            outv[:, 0:jh], Alu.min)
                    nc.vector.tensor_tensor(outv[:, jh:N_J], tmpv[:, jh:N_J],
                                            outv[:, jh:N_J], Alu.min)
                else:
                    nc.vector.tensor_tensor(outv, tmpv, outv, Alu.min)

        d2tv = d2t_d.rearrange("(j p) b h -> p j (b h)", p=P)
        accv = ACC[:].rearrange("p (j c) -> p j c", j=N_J)
        jh = N_J // 2
        nc.sync.dma_start(d2tv[:, 0:jh], accv[:, 0:jh])
        nc.scalar.dma_start(d2tv[:, jh:N_J], accv[:, jh:N_J])


HB = 8                    # input halo rows per side (>= R)
HE = HS + 2 * HB          # 80 extended rows per core
N_T3 = (B * HE) // P      # 10 scan tiles


def _k3_body(tc, out_d, xs_d):
    """Fused single-launch EDT: W-scans + transpose/square + banded H-pass +
    banded B-pass + sqrt.  Needs only an input halo of HB >= R rows (host
    pads with foreground), so no cross-core communication at all.

    xs_d:  [16, HE, 512] f32 dram (ExternalInput, host-padded h-slab)
    out_d: [512, 16, HS] f32 dram (ExternalOutput), distances, w-major
    """
    import concourse.mybir as mybir

    nc = tc.nc
    f16 = mybir.dt.float16
    f32 = mybir.dt.float32
    Alu = mybir.AluOpType
    Act = mybir.ActivationFunctionType
    N_J = W // P              # 4 w-groups
    CE = B * HE               # 1280 lines, also transposed free size per j
    C = B * HS                # 1024 interior (b,h) elements per w

    from concourse.masks import make_identity

    with tc.tile_pool(name="const", bufs=1) as cpool, \
         tc.tile_pool(name="big", bufs=1) as bpool, \
         tc.tile_pool(name="tmp", bufs=3) as tpool, \
         tc.tile_pool(name="psum", bufs=3, space="PSUM") as ppool, \
         tc.tile_pool(name="psumw", bufs=1, space="PSUM") as ppoolw:

        ident = cpool.tile([P, P], f16)
        make_identity(nc, ident[:])
        ones = cpool.tile([P, W], f16)
        nc.vector.memset(ones[:], 1.0)
        psw = ppoolw.tile([P, P], f16)
        nc.tensor.transpose(psw[:], ident[:], ident[:])

        XH = bpool.tile([P, N_T3 * W], f16)
        AALL = bpool.tile([P, N_T3 * W], f16)
        FALL = bpool.tile([P, N_T3 * W], f16)
        DALL = bpool.tile([P, N_T3 * W], f16)

        xflat = xs_d.rearrange("b h w -> (b h) w")
        for m in range(5):
            nc.gpsimd.dma_start(
                XH[:, 2 * W * m: 2 * W * (m + 1)].rearrange(
                    "p (g w) -> p g w", g=2),
                xflat[256 * m: 256 * (m + 1)].rearrange(
                    "(g pp) w -> pp g w", g=2))
        for m in range(5):
            nc.vector.tensor_scalar(
                AALL[:, 2 * W * m: 2 * W * (m + 1)],
                XH[:, 2 * W * m: 2 * W * (m + 1)],
                0.0, CLAMP, Alu.not_equal, Alu.mult)

        def seg(t):
            return W * t

        for t in range(N_T3):
            fa = FALL[:, W * t: W * (t + 1)]
            nc.vector.tensor_tensor_scan(
                fa, ones[:, 0:W], AALL[:, W * t: W * (t + 1)], CLAMP,
                Alu.add, Alu.min)
            nc.vector.tensor_tensor_scan(
                DALL[:, W * t: W * (t + 1)][:, ::-1], ones[:, 0:W],
                fa[:, ::-1], CLAMP, Alu.add, Alu.min)

        SQ = bpool.tile([P, N_J * CE], f16)     # w lines x (j, b, h80)
        for j in range(N_J):
            # two PSUM tiles per j: 640 fp16 = 1280B stays inside one 2KB
            # PSUM bank (a straddling AP faults the exec unit)
            for hf in range(2):
                ps = ppool.tile([P, CE // 2], f16, tag="ps")
                for tt_ in range(N_T3 // 2):
                    t = hf * (N_T3 // 2) + tt_
                    nc.tensor.transpose(
                        ps[:, P * tt_:P * (tt_ + 1)],
                        DALL[:, seg(t) + P * j: seg(t) + P * (j + 1)],
                        ident[:])
                nc.scalar.activation(
                    SQ[:, CE * j + (CE // 2) * hf:
                       CE * j + (CE // 2) * (hf + 1)], ps[:], Act.Square)

        # SQB[c] = SQ[c+1]: 4B-aligned source for odd H-shifts
        SQB = bpool.tile([P, N_J * CE], f16)
        nc.scalar.activation(SQB[:, 0:N_J * CE - 1], SQ[:, 1:N_J * CE],
                             Act.Copy, bias=0.0)

        # banded pass H: pair = min(left, right), tmp = pair + s^2,
        # accH = min(accH, tmp).  Output interior h in [HB, HB+HS).
        ACH = bpool.tile([P, N_J * C], f16)
        sq5 = SQ[:].rearrange("p (j b h) -> p j b h", j=N_J, b=B)
        sqb5 = SQB[:].rearrange("p (j b h) -> p j b h", j=N_J, b=B)
        ah4 = ACH[:].rearrange("p (j b h) -> p j b h", j=N_J, b=B)
        center = sq5[:, :, :, HB:HB + HS]
        order = sorted(range(1, R + 1), key=lambda s: (s % 2, -s))
        for si, s in enumerate(order):
            if s % 2 == 0:
                lo = sq5[:, :, :, HB - s:HB - s + HS]
                hi = sq5[:, :, :, HB + s:HB + s + HS]
            else:
                lo = sqb5[:, :, :, HB - s - 1:HB - s - 1 + HS]
                hi = sqb5[:, :, :, HB + s - 1:HB + s - 1 + HS]
            pair = tpool.tile([P, N_J * C], f16, tag="pair")
            pair4 = pair[:].rearrange("p (j b h) -> p j b h", j=N_J, b=B)
            if si == 0:
                jh = N_J // 2
                nc.vector.tensor_tensor(pair4[:, 0:jh], lo[:, 0:jh],
                                        hi[:, 0:jh], Alu.min)
                nc.vector.tensor_tensor(pair4[:, jh:N_J], lo[:, jh:N_J],
                                        hi[:, jh:N_J], Alu.min)
            else:
                nc.vector.tensor_tensor(pair4, lo, hi, Alu.min)
            tmp = tpool.tile([P, N_J * C], f16, tag="tmp")
            tmp4 = tmp[:].rearrange("p (j b h) -> p j b h", j=N_J, b=B)
            if s == 2:
                nc.vector.tensor_scalar(tmp4, pair4, float(s * s), None,
                                        Alu.add)
            else:
                nc.scalar.activation(tmp4, pair4, Act.Copy, bias=float(s * s))
            if si == 0:
                nc.vector.tensor_tensor(ah4, tmp4, center, Alu.min)
            else:
                nc.vector.tensor_tensor(ah4, tmp4, ah4, Alu.min)

        # banded pass B on the compact (j, b, h64) layout
        ACC = bpool.tile([P, N_J * C], f16)
        ac4 = ACC[:].rearrange("p (j b h) -> p j b h", j=N_J, b=B)
        nc.scalar.activation(ac4[:, :, B - 1:B, :], ah4[:, :, B - 1:B, :],
                             Act.Copy, bias=0.0)
        first = True
        for s in range(1, R + 1):
            bc = B - s
            for sgn in (1, -1):
                if sgn > 0:
                    srcv = ah4[:, :, s:s + bc, :]
                    outv = ac4[:, :, 0:bc, :]
                else:
                    srcv = ah4[:, :, 0:bc, :]
                    outv = ac4[:, :, s:B, :]
                tmp = tpool.tile([P, N_J * C], f16, tag="tmpb")
                tmpv = tmp[:].rearrange(
                    "p (j b h) -> p j b h", j=N_J, b=B)[:, :, 0:bc, :]
                if s >= 2:
                    nc.scalar.activation(tmpv, srcv, Act.Copy,
                                         bias=float(s * s))
                else:
                    nc.vector.tensor_scalar(tmpv, srcv, float(s * s), None,
                                            Alu.add)
                if first:
                    nc.vector.tensor_tensor(outv, tmpv, ah4[:, :, 0:bc, :],
                                            Alu.min)
                    first = False
                elif s == R and sgn == -1:
                    for j in range(N_J):
                        nc.vector.tensor_tensor(
                            outv[:, j:j + 1], tmpv[:, j:j + 1],
                            outv[:, j:j + 1], Alu.min)
                else:
                    nc.vector.tensor_tensor(outv, tmpv, outv, Alu.min)

        OUT = bpool.tile([P, N_J * C], f32)
        outd = out_d.rearrange("(j p) b h -> p j (b h)", p=P)
        outs = OUT[:].rearrange("p (j c) -> p j c", j=N_J)
        accs = ACC[:].rearrange("p (j c) -> p j c", j=N_J)
        for j in range(N_J):
            nc.scalar.activation(outs[:, j:j + 1], accs[:, j:j + 1], Act.Sqrt)
            eng = nc.sync if j % 2 == 0 else nc.scalar
            eng.dma_start(outd[:, j:j + 1], outs[:, j:j + 1])


def _k2_body(tc, out_d, d2s_d):
    """Banded pass H + sqrt.

    d2s_d: [WS, 16, 512] f16 dram (ExternalInput)
    out_d: [WS, 16, 512] f32 dram (ExternalOutput), distances

    Row-padded layout (PADH pad columns, value 2047 == +inf) so every
    shifted operand is full-width; odd shifts read SRCB (CALL displaced by
    one column) so all hot operands stay 4B-aligned.  Per |s| the two
    directions fold as pair = min(left, right) (DVE), tmp = pair + s^2
    (ACT, or DVE tensor_scalar for one shift), acc = min(acc, tmp) (DVE).
    """
    import concourse.mybir as mybir

    nc = tc.nc
    f16 = mybir.dt.float16
    f32 = mybir.dt.float32
    Alu = mybir.AluOpType
    Act = mybir.ActivationFunctionType
    N_G = (WS * B) // P       # 8 groups of 128 (w,b) lines
    PADH = 8
    HP = H + 2 * PADH         # 528 padded row width

    with tc.tile_pool(name="big", bufs=1) as bpool, \
         tc.tile_pool(name="tmp", bufs=3) as tpool:

        CALL = bpool.tile([P, N_G * HP], f16)
        SRCB = bpool.tile([P, N_G * HP], f16)
        ACC = bpool.tile([P, N_G * HP], f16)
        # pad strips (+inf) for CALL rows [0:PADH) and [PADH+H:HP),
        # for SRCB rows [0:PADH-1) and [PADH-1+H:HP)
        cstrips = CALL[:].rearrange("p (g h) -> p g h", g=N_G)
        nc.gpsimd.memset(cstrips[:, :, 0:PADH], 2047.0)
        nc.gpsimd.memset(cstrips[:, :, PADH + H:HP], 2047.0)
        strips = SRCB[:].rearrange("p (g h) -> p g h", g=N_G)
        nc.gpsimd.memset(strips[:, :, 0:PADH - 1], 2047.0)
        nc.gpsimd.memset(strips[:, :, PADH - 1 + H:HP], 2047.0)

        call3 = CALL[:].rearrange("p (g h) -> p g h", g=N_G)
        srcb3 = SRCB[:].rearrange("p (g h) -> p g h", g=N_G)
        acc3 = ACC[:].rearrange("p (g h) -> p g h", g=N_G)
        accint = acc3[:, :, PADH:PADH + H]
        d2sv = d2s_d.rearrange("(g ww) b h -> (ww b) g (h)",
                               g=N_G, ww=WS // N_G)
        # three copies of the input land by DMA: source, 1-column-shifted
        # source (odd-shift alignment helper), and the accumulator init.
        nc.sync.dma_start(call3[:, :, PADH:PADH + H], d2sv)
        nc.scalar.dma_start(srcb3[:, :, PADH - 1:PADH - 1 + H], d2sv)
        nc.sync.dma_start(accint, d2sv)

        order = sorted(range(1, R + 1), key=lambda s: (s % 2, -s))
        assert order[-1] == 1
        for s in order:
            if s % 2 == 0:
                lo = call3[:, :, PADH - s:PADH - s + H]
                hi = call3[:, :, PADH + s:PADH + s + H]
            else:
                lo = srcb3[:, :, PADH - s - 1:PADH - s - 1 + H]
                hi = srcb3[:, :, PADH + s - 1:PADH + s - 1 + H]
            pair = tpool.tile([P, N_G * H], f16, tag="pair")
            pair3 = pair[:].rearrange("p (g h) -> p g h", g=N_G)
            nc.vector.tensor_tensor(pair3, lo, hi, Alu.min)
            tmp = tpool.tile([P, N_G * H], f16, tag="tmp")
            tmp3 = tmp[:].rearrange("p (g h) -> p g h", g=N_G)
            if s == 2:
                nc.vector.tensor_scalar(tmp3, pair3, float(s * s), None,
                                        Alu.add)
            else:
                nc.scalar.activation(tmp3, pair3, Act.Copy, bias=float(s * s))
            if s == 1:
                # split the chain tail so sqrt + store of the first half
                # overlap the second half's min
                hh = N_G // 2
                nc.vector.tensor_tensor(accint[:, 0:hh], tmp3[:, 0:hh],
                                        accint[:, 0:hh], Alu.min)
                nc.vector.tensor_tensor(accint[:, hh:N_G], tmp3[:, hh:N_G],
                                        accint[:, hh:N_G], Alu.min)
            else:
                nc.vector.tensor_tensor(accint, tmp3, accint, Alu.min)

        OUT = bpool.tile([P, N_G * H], f32)
        out3 = OUT[:].rearrange("p (g h) -> p g h", g=N_G)
        outv = out_d.rearrange("(g ww) b h -> (ww b) g (h)",
                               g=N_G, ww=WS // N_G)
        hh = N_G // 2
        for half in range(2):
            sl = slice(hh * half, hh * (half + 1))
            nc.scalar.activation(out3[:, sl], accint[:, sl], Act.Sqrt)
            eng = nc.sync if half == 0 else nc.scalar
            eng.dma_start(outv[:, sl], out3[:, sl])


def _split_multi_waits(nc):
    """Walrus in this toolchain encodes at most ONE sync wait per hardware
    instruction.  Hoist extra waits onto same-engine NoOp carriers inserted
    immediately before the over-subscribed instruction (program order on the
    engine preserves the semantics exactly)."""
    import concourse.mybir as mybir

    n = 0
    for fn in nc.m.functions:
        for blk in fn.blocks:
            insts = blk.instructions
            out = []
            for inst in insts:
                si = inst.sync_info
                if si is not None and len(si.on_wait) > 1:
                    waits = list(si.on_wait)
                    for w in waits[:-1]:
                        nop = mybir.InstNoOp(
                            name=f"waitsplit-{n}", ins=[], outs=[])
                        n += 1
                        nop.engine = inst.engine
                        nop.sync_info = mybir.SyncInfo(
                            on_wait=[w], on_update=[])
                        out.append(nop)
                    inst.sync_info = mybir.SyncInfo(
                        on_wait=[waits[-1]], on_update=list(si.on_update))
                out.append(inst)
            blk.instructions = out
    return n


def _make_tc_class():
    """TileContext whose kernel-tail drain is split into one drain per proc.

    The stock tail emits a single sync-engine Drain waiting on every
    outstanding processor; this walrus build only encodes ONE sync wait per
    instruction, so the aggregated drain fails codegen.  Semantics are
    identical — the waits just land on consecutive Drain instructions.
    """
    import concourse.tile as tile
    from concourse.vector_clock import ScopedClock, VectorClock

    class SplitDrainTileContext(tile.TileContext):
        def _drain_and_barrier(self, tick_clock, wait_clock):
            gvc = tick_clock.global_clock
            for proc in range(len(gvc)):
                t = gvc[proc]
                if t <= 0:
                    continue
                d = self.nc.sync.drain()
                sv = VectorClock([0] * len(gvc))
                sv.require_at_least(proc, t)
                wait_clock.add_sem_waits(d.ins, ScopedClock({None: sv}))
            self.nc.all_engine_barrier()
            assert self.sems is not None
            popped = self.nc._tile_sem_poison_stack.pop()
            assert popped is self._sem_poison
            self.nc.clear_and_free_semaphores(
                list(self.sems.allocated().values()))
            self.nc.all_engine_barrier()

    return SplitDrainTileContext


def _build():
    """Build the fused Bass module (done once per process)."""
    import concourse.bass as bass
    import concourse.mybir as mybir

    f16 = mybir.dt.float16
    f32 = mybir.dt.float32
    TC = _make_tc_class()

    nc3 = bass.Bass("TRN2", debug=False, num_devices=NCORES)
    xs3_d = nc3.dram_tensor("xs3", [B, HE, W], f32,
                            kind="ExternalInput").ap()
    ot3_d = nc3.dram_tensor("ot3", [W, B, HS], f32,
                            kind="ExternalOutput").ap()
    with TC(nc3) as tc:
        _k3_body(tc, ot3_d, xs3_d)
    _split_multi_waits(nc3)
    return (nc3,)


def _build_two_launch():
    """Older two-launch pipeline (kept as reference/fallback)."""
    import concourse.bass as bass
    import concourse.mybir as mybir

    f16 = mybir.dt.float16
    f32 = mybir.dt.float32
    TC = _make_tc_class()

    nc1 = bass.Bass("TRN2", debug=False, num_devices=NCORES)
    xs_d = nc1.dram_tensor("xs", [B, HS, W], f32, kind="ExternalInput").ap()
    d2t_d = nc1.dram_tensor("d2t", [W, B, HS], f16, kind="ExternalOutput").ap()
    with TC(nc1) as tc:
        _k1_body(tc, d2t_d, xs_d)
    _split_multi_waits(nc1)

    nc2 = bass.Bass("TRN2", debug=False, num_devices=NCORES)
    d2s_d = nc2.dram_tensor("d2s", [WS, B, H], f16, kind="ExternalInput").ap()
    out_d = nc2.dram_tensor("ot", [WS, B, H], f32, kind="ExternalOutput").ap()
    with TC(nc2) as tc:
        _k2_body(tc, out_d, d2s_d)
    _split_multi_waits(nc2)

    return nc1, nc2


def _host_exact_edt(x):
    """Exact host fallback: banded numpy EDT with growing radius (f32)."""
    INF = np.float32(1e9)
    r = 2 * R
    while True:
        d0 = np.where(x != 0, INF, np.float32(0.0))
        fwd = np.empty_like(d0)
        st = np.full(d0.shape[:2], INF, np.float32)
        for w in range(W):
            st = np.minimum(st + 1.0, d0[:, :, w]); fwd[:, :, w] = st
        st = np.full(d0.shape[:2], INF, np.float32)
        bwd = np.empty_like(d0)
        for w in range(W - 1, -1, -1):
            st = np.minimum(st + 1.0, d0[:, :, w]); bwd[:, :, w] = st
        d2 = np.minimum(fwd, bwd) ** 2
        for axis in (0, 1):
            src = d2
            acc = src.copy()
            rr = min(r, x.shape[axis] - 1)
            for s in range(1, rr + 1):
                sl_lo = [slice(None)] * 3
                sl_hi = [slice(None)] * 3
                sl_lo[axis] = slice(0, x.shape[axis] - s)
                sl_hi[axis] = slice(s, None)
                np.minimum(acc[tuple(sl_lo)], src[tuple(sl_hi)] + s * s,
                           out=acc[tuple(sl_lo)])
                np.minimum(acc[tuple(sl_hi)], src[tuple(sl_lo)] + s * s,
                           out=acc[tuple(sl_hi)])
            d2 = acc
        out = np.sqrt(d2)
        # exact when every per-axis offset fits in the band; r >= max dim
        # means the bands are complete regardless of the value of out
        if out.max() <= r or r >= max(x.shape):
            return out.astype(np.float32)
        r *= 2


def kernel(x):
    global _BUILT
    x = np.asarray(x)
    assert x.shape == (B, H, W)
    if x.dtype != np.float32:
        x = x.astype(np.float32)

    from concourse.bass_utils import run_bass_kernel_spmd

    if _BUILT is None:
        _BUILT = _build()
    (nc3,) = _BUILT
    LAST_RESULTS.clear()

    core_ids = list(range(NCORES))
    # pad H with foreground (nonzero) halo rows; each core gets its slab
    # plus HB halo rows each side, so all three passes are core-local.
    xp = np.pad(x, ((0, 0), (HB, HB), (0, 0)), constant_values=1.0)
    in3 = [{"xs3": np.ascontiguousarray(xp[:, k * HS:k * HS + HE, :])}
           for k in range(NCORES)]
    r3 = run_bass_kernel_spmd(nc3, in3, core_ids)
    LAST_RESULTS.append(r3)
    outt = np.concatenate([r3.results[k]["ot3"] for k in range(NCORES)],
                          axis=2)

    out = outt.transpose(1, 2, 0)          # (w,b,h) -> (b,h,w)

    # Banding is exact iff the true max distance < R + 1 (per-axis integer
    # offsets of the optimal zero are bounded by floor of the distance, and
    # the banded result upper-bounds the true one).
    if float(np.max(out)) >= R + 1:
        out = _host_exact_edt(x)

    nan_mask = np.isnan(x)
    if nan_mask.any():
        out = np.where(nan_mask, np.float32(np.nan), out)
    return out


# revision 36
# speedup vs baseline: 1.0317x; 1.0180x over previous
"""Trainium2 Bass kernel: exact 3D Euclidean distance transform of a binary
(16, 512, 512) float32 volume — distance from every nonzero voxel to the
nearest zero voxel over ALL three axes (batch participates in the metric),
matching scipy.ndimage.distance_transform_edt on the full array.

Algorithm (separable min-plus with squared-parabola kernels; order-free):
  pass W: exact 1D nearest-zero distance along W via fwd/bwd saturating
          scans (tensor_tensor_scan, values clamp at CLAMP=32 == +inf),
          then square (PE transpose + ACT Square).
  pass B: banded parabola min-plus along B (|delta| <= R).
  pass H: banded parabola min-plus along H (|delta| <= R).
Banding is exact whenever the true max distance <= R; this is verified on
the host after the run (max over the output), with an exact host fallback
otherwise.  All intermediates are small integers (<= CLAMP^2 + R^2), exact
in fp16, which unlocks the DVE 2x perf mode.

Sharding: kernel 1 is data-parallel over H (8 slabs of 64 rows; the W-scan
and the B-pass need full W and full B, which each slab has).  The squared
intermediate is stored w-major; the host reshards it W-wise (numpy slicing)
and kernel 2 (data-parallel over W, 8 slabs of 64 cols) runs the H-pass,
which needs full H, plus the final sqrt.

Hardware quirk driving the structure: several instruction encodings
(DMA DIRECT2D, the S2S2D2 tensor-scalar-ptr family used by the scans and
scalar_tensor_tensor) accept only ONE semaphore wait.  Tile emits a wait
only when an engine's vector clock is behind, so the kernel is arranged so
every such instruction has at most one not-yet-observed cross-engine
dependency, and each kernel issues at most 8 HWDGE DMAs (no lane reuse).
"""
import numpy as np

B, H, W = 16, 512, 512
NCORES = 8
HS = H // NCORES
WS = W // NCORES
P = 128
CLAMP = 32.0
R = 5

_BUILT = None
LAST_RESULTS = []   # BassKernelResults of the most recent kernel() call


def _k1_body(tc, d2t_d, xs_d):
    """Pass W scans + PE transpose/ACT square + banded pass B.

    xs_d:  [16, HS, 512] f32 dram (ExternalInput)
    d2t_d: [512, 16, HS] f16 dram (ExternalOutput), squared distances

    Engine balance: both scans run on DVE (the bwd scan runs on the fwd
    result, which already equals min(fwd, d0), so its output is
    min(fwd, bwd) directly — no separate combine).  The banded pass-B adds
    are split between ACT (Copy+bias) and DVE (tensor_scalar); the mins run
    on DVE (tensor_tensor, 2x mode: every operand offset is a multiple of
    HS=64 fp16 elements, so alignment holds).
    """
    import concourse.mybir as mybir

    nc = tc.nc
    f16 = mybir.dt.float16
    f32 = mybir.dt.float32
    Alu = mybir.AluOpType
    Act = mybir.ActivationFunctionType
    N_T = (B * HS) // P       # 8 scan tiles
    N_J = W // P              # 4 w-groups
    C = B * HS                # 1024 free elements per w after transpose

    from concourse.masks import make_identity

    with tc.tile_pool(name="const", bufs=1) as cpool, \
         tc.tile_pool(name="big", bufs=1) as bpool, \
         tc.tile_pool(name="tmp", bufs=3) as tpool, \
         tc.tile_pool(name="psum", bufs=7, space="PSUM") as ppool, \
         tc.tile_pool(name="psumw", bufs=1, space="PSUM") as ppoolw:

        ident = cpool.tile([P, P], f16)
        make_identity(nc, ident[:])
        ones = cpool.tile([P, W], f16)
        nc.vector.memset(ones[:], 1.0)
        # dummy transpose so PE observes the gpsimd-built identity before the
        # real transposes (keeps every matmul at <= 1 semaphore wait)
        psw = ppoolw.tile([P, P], f16)
        nc.tensor.transpose(psw[:], ident[:], ident[:])

        XH = bpool.tile([P, N_T * W], f16)      # x cast to f16 by the DMA
        AALL = bpool.tile([P, N_T * W], f16)    # d0 = (x != 0) * CLAMP
        FALL = bpool.tile([P, N_T * W], f16)    # fwd scan
        DALL = bpool.tile([P, N_T * W], f16)    # bwd scan of fwd = 1D dist

        qs = N_T * W // 4
        for m in range(4):
            nc.gpsimd.dma_start(
                XH[:, qs * m: qs * (m + 1)].rearrange(
                    "p (g w) -> p g w", g=2),
                xs_d[4 * m: 4 * (m + 1)].rearrange(
                    "(g bb) h w -> (bb h) g w", g=2, bb=2))
        for q in range(4):
            qs = N_T * W // 4
            nc.vector.tensor_scalar(
                AALL[:, qs * q: qs * (q + 1)],
                XH[:, qs * q: qs * (q + 1)],
                0.0, CLAMP, Alu.not_equal, Alu.mult)

        for t in range(N_T):
            fa = FALL[:, W * t: W * (t + 1)]
            nc.vector.tensor_tensor_scan(
                fa, ones[:], AALL[:, W * t: W * (t + 1)], CLAMP,
                Alu.add, Alu.min)
            nc.vector.tensor_tensor_scan(
                DALL[:, W * t: W * (t + 1)][:, ::-1], ones[:], fa[:, ::-1],
                CLAMP, Alu.add, Alu.min)

        SQ = bpool.tile([P, N_J * C], f16)      # w lines x (j, b, h)
        for j in range(N_J):
            ps = ppool.tile([P, C], f16, tag="ps")
            for t in range(N_T):
                nc.tensor.transpose(
                    ps[:, P * t:P * (t + 1)],
                    DALL[:, W * t + P * j: W * t + P * (j + 1)], ident[:])
            nc.scalar.activation(SQ[:, C * j:C * (j + 1)], ps[:], Act.Square)

        # banded pass B: per shift, an add (ACT or DVE tensor_scalar) then a
        # DVE tensor_tensor min.  ACC is initialized by the first (s=1) min
        # for b<15 plus a small ACT copy for the b=15 strip.
        ACC = bpool.tile([P, N_J * C], f16)
        sq4 = SQ[:].rearrange("p (j b h) -> p j b h", j=N_J, b=B)
        ac4 = ACC[:].rearrange("p (j b h) -> p j b h", j=N_J, b=B)
        nc.scalar.activation(ac4[:, :, B - 1:B, :], sq4[:, :, B - 1:B, :],
                             Act.Copy, bias=0.0)
        first = True
        for s in range(1, R + 1):
            bc = B - s
            for sgn in (1, -1):
                if sgn > 0:
                    srcv = sq4[:, :, s:s + bc, :]
                    outv = ac4[:, :, 0:bc, :]
                else:
                    srcv = sq4[:, :, 0:bc, :]
                    outv = ac4[:, :, s:B, :]
                tmp = tpool.tile([P, N_J * C], f16, tag="tmp")
                tmpv = tmp[:].rearrange(
                    "p (j b h) -> p j b h", j=N_J, b=B)[:, :, 0:bc, :]
                if s >= 2:
                    nc.scalar.activation(tmpv, srcv, Act.Copy,
                                         bias=float(s * s))
                else:
                    nc.vector.tensor_scalar(tmpv, srcv, float(s * s), None,
                                            Alu.add)
                if first:
                    # acc := min(sq (delta=0), tmp) initializes b < 15
                    nc.vector.tensor_tensor(outv, tmpv, sq4[:, :, 0:bc, :],
                                            Alu.min)
                    first = False
                elif s == R and sgn == -1:
                    # split the final fold by j-halves so the two output DMAs
                    # can start as soon as their half is done
                    jh = N_J // 2
                    nc.vector.tensor_tensor(outv[:, 0:jh], tmpv[:, 0:jh],
                                            outv[:, 0:jh], Alu.min)
                    nc.vector.tensor_tensor(outv[:, jh:N_J], tmpv[:, jh:N_J],
                                            outv[:, jh:N_J], Alu.min)
                else:
                    nc.vector.tensor_tensor(outv, tmpv, outv, Alu.min)

        d2tv = d2t_d.rearrange("(j p) b h -> p j (b h)", p=P)
        accv = ACC[:].rearrange("p (j c) -> p j c", j=N_J)
        jh = N_J // 2
        nc.sync.dma_start(d2tv[:, 0:jh], accv[:, 0:jh])
        nc.scalar.dma_start(d2tv[:, jh:N_J], accv[:, jh:N_J])


HB = 8                    # input halo rows per side (>= R)
HE = HS + 2 * HB          # 80 extended rows per core
N_T3 = (B * HE) // P      # 10 scan tiles


def _k3_body(tc, out_d, xs_d):
    """Fused single-launch EDT: W-scans + transpose/square + banded H-pass +
    banded B-pass + sqrt.  Needs only an input halo of HB >= R rows (host
    pads with foreground), so no cross-core communication at all.

    xs_d:  [16, HE, 512] f32 dram (ExternalInput, host-padded h-slab)
    out_d: [512, 16, HS] f32 dram (ExternalOutput), distances, w-major
    """
    import concourse.mybir as mybir

    nc = tc.nc
    f16 = mybir.dt.float16
    f32 = mybir.dt.float32
    Alu = mybir.AluOpType
    Act = mybir.ActivationFunctionType
    N_J = W // P              # 4 w-groups
    CE = B * HE               # 1280 lines, also transposed free size per j
    C = B * HS                # 1024 interior (b,h) elements per w

    from concourse.masks import make_identity

    with tc.tile_pool(name="const", bufs=1) as cpool, \
         tc.tile_pool(name="big", bufs=1) as bpool, \
         tc.tile_pool(name="tmp", bufs=4) as tpool, \
         tc.tile_pool(name="psum", bufs=6, space="PSUM") as ppool, \
         tc.tile_pool(name="psumw", bufs=1, space="PSUM") as ppoolw:

        ident = cpool.tile([P, P], f16)
        make_identity(nc, ident[:])
        ones = cpool.tile([P, W], f16)
        nc.vector.memset(ones[:], 1.0)
        psw = ppoolw.tile([P, P], f16)
        nc.tensor.transpose(psw[:], ident[:], ident[:])

        XH = bpool.tile([P, N_T3 * W], f16)
        AALL = bpool.tile([P, N_T3 * W], f16)
        FALL = bpool.tile([P, N_T3 * W], f16)
        DALL = bpool.tile([P, N_T3 * W], f16)

        xflat = xs_d.rearrange("b h w -> (b h) w")
        for m in range(5):
            nc.gpsimd.dma_start(
                XH[:, 2 * W * m: 2 * W * (m + 1)].rearrange(
                    "p (g w) -> p g w", g=2),
                xflat[256 * m: 256 * (m + 1)].rearrange(
                    "(g pp) w -> pp g w", g=2))
        for m in range(5):
            nc.vector.tensor_scalar(
                AALL[:, 2 * W * m: 2 * W * (m + 1)],
                XH[:, 2 * W * m: 2 * W * (m + 1)],
                0.0, CLAMP, Alu.not_equal, Alu.mult)

        def seg(t):
            return W * t

        for t in range(N_T3):
            fa = FALL[:, W * t: W * (t + 1)]
            nc.vector.tensor_tensor_scan(
                fa, ones[:, 0:W], AALL[:, W * t: W * (t + 1)], CLAMP,
                Alu.add, Alu.min)
            nc.vector.tensor_tensor_scan(
                DALL[:, W * t: W * (t + 1)][:, ::-1], ones[:, 0:W],
                fa[:, ::-1], CLAMP, Alu.add, Alu.min)

        SQ = bpool.tile([P, N_J * CE], f16)     # w lines x (j, b, h80)
        for j in range(N_J):
            # two PSUM tiles per j: 640 fp16 = 1280B stays inside one 2KB
            # PSUM bank (a straddling AP faults the exec unit)
            for hf in range(2):
                ps = ppool.tile([P, CE // 2], f16, tag="ps")
                for tt_ in range(N_T3 // 2):
                    t = hf * (N_T3 // 2) + tt_
                    nc.tensor.transpose(
                        ps[:, P * tt_:P * (tt_ + 1)],
                        DALL[:, seg(t) + P * j: seg(t) + P * (j + 1)],
                        ident[:])
                nc.scalar.activation(
                    SQ[:, CE * j + (CE // 2) * hf:
                       CE * j + (CE // 2) * (hf + 1)], ps[:], Act.Square)

        # SQB[c] = SQ[c+1]: 4B-aligned source for odd H-shifts
        SQB = bpool.tile([P, N_J * CE], f16)
        nc.scalar.activation(SQB[:, 0:N_J * CE - 1], SQ[:, 1:N_J * CE],
                             Act.Copy, bias=0.0)

        # banded pass H: pair = min(left, right), tmp = pair + s^2,
        # accH = min(accH, tmp).  Output interior h in [HB, HB+HS).
        ACH = bpool.tile([P, N_J * C], f16)
        sq5 = SQ[:].rearrange("p (j b h) -> p j b h", j=N_J, b=B)
        sqb5 = SQB[:].rearrange("p (j b h) -> p j b h", j=N_J, b=B)
        ah4 = ACH[:].rearrange("p (j b h) -> p j b h", j=N_J, b=B)
        center = sq5[:, :, :, HB:HB + HS]
        order = sorted(range(1, R + 1), key=lambda s: (s % 2, -s))
        for si, s in enumerate(order):
            if s % 2 == 0:
                lo = sq5[:, :, :, HB - s:HB - s + HS]
                hi = sq5[:, :, :, HB + s:HB + s + HS]
            else:
                lo = sqb5[:, :, :, HB - s - 1:HB - s - 1 + HS]
                hi = sqb5[:, :, :, HB + s - 1:HB + s - 1 + HS]
            pair = tpool.tile([P, N_J * C], f16, tag="pair")
            pair4 = pair[:].rearrange("p (j b h) -> p j b h", j=N_J, b=B)
            if si == 0:
                jh = N_J // 2
                nc.vector.tensor_tensor(pair4[:, 0:jh], lo[:, 0:jh],
                                        hi[:, 0:jh], Alu.min)
                nc.vector.tensor_tensor(pair4[:, jh:N_J], lo[:, jh:N_J],
                                        hi[:, jh:N_J], Alu.min)
            else:
                nc.vector.tensor_tensor(pair4, lo, hi, Alu.min)
            tmp = tpool.tile([P, N_J * C], f16, tag="tmp")
            tmp4 = tmp[:].rearrange("p (j b h) -> p j b h", j=N_J, b=B)
            if s == 2:
                nc.vector.tensor_scalar(tmp4, pair4, float(s * s), None,
                                        Alu.add)
            else:
                nc.scalar.activation(tmp4, pair4, Act.Copy, bias=float(s * s))
            if si == 0:
                nc.vector.tensor_tensor(ah4, tmp4, center, Alu.min)
            else:
                nc.vector.tensor_tensor(ah4, tmp4, ah4, Alu.min)

        # banded pass B on the compact (j, b, h64) layout
        ACC = bpool.tile([P, N_J * C], f16)
        ac4 = ACC[:].rearrange("p (j b h) -> p j b h", j=N_J, b=B)
        nc.scalar.activation(ac4[:, :, B - 1:B, :], ah4[:, :, B - 1:B, :],
                             Act.Copy, bias=0.0)
        first = True
        for s in range(1, R + 1):
            bc = B - s
            for sgn in (1, -1):
                if sgn > 0:
                    srcv = ah4[:, :, s:s + bc, :]
                    outv = ac4[:, :, 0:bc, :]
                else:
                    srcv = ah4[:, :, 0:bc, :]
                    outv = ac4[:, :, s:B, :]
                tmp = tpool.tile([P, N_J * C], f16, tag="tmpb")
                tmpv = tmp[:].rearrange(
                    "p (j b h) -> p j b h", j=N_J, b=B)[:, :, 0:bc, :]
                if s >= 2:
                    nc.scalar.activation(tmpv, srcv, Act.Copy,
                                         bias=float(s * s))
                else:
                    nc.vector.tensor_scalar(tmpv, srcv, float(s * s), None,
                                            Alu.add)
                if first:
                    nc.vector.tensor_tensor(outv, tmpv, ah4[:, :, 0:bc, :],
                                            Alu.min)
                    first = False
                elif s == R and sgn == -1:
                    for j in range(N_J):
                        nc.vector.tensor_tensor(
                            outv[:, j:j + 1], tmpv[:, j:j + 1],
                            outv[:, j:j + 1], Alu.min)
                else:
                    nc.vector.tensor_tensor(outv, tmpv, outv, Alu.min)

        OUT = bpool.tile([P, N_J * C], f32)
        outd = out_d.rearrange("(j p) b h -> p j (b h)", p=P)
        outs = OUT[:].rearrange("p (j c) -> p j c", j=N_J)
        accs = ACC[:].rearrange("p (j c) -> p j c", j=N_J)
        for j in range(N_J):
            nc.scalar.activation(outs[:, j:j + 1], accs[:, j:j + 1], Act.Sqrt)
            eng = nc.sync if j % 2 == 0 else nc.scalar
            eng.dma_start(outd[:, j:j + 1], outs[:, j:j + 1])


def _k2_body(tc, out_d, d2s_d):
    """Banded pass H + sqrt.

    d2s_d: [WS, 16, 512] f16 dram (ExternalInput)
    out_d: [WS, 16, 512] f32 dram (ExternalOutput), distances

    Row-padded layout (PADH pad columns, value 2047 == +inf) so every
    shifted operand is full-width; odd shifts read SRCB (CALL displaced by
    one column) so all hot operands stay 4B-aligned.  Per |s| the two
    directions fold as pair = min(left, right) (DVE), tmp = pair + s^2
    (ACT, or DVE tensor_scalar for one shift), acc = min(acc, tmp) (DVE).
    """
    import concourse.mybir as mybir

    nc = tc.nc
    f16 = mybir.dt.float16
    f32 = mybir.dt.float32
    Alu = mybir.AluOpType
    Act = mybir.ActivationFunctionType
    N_G = (WS * B) // P       # 8 groups of 128 (w,b) lines
    PADH = 8
    HP = H + 2 * PADH         # 528 padded row width

    with tc.tile_pool(name="big", bufs=1) as bpool, \
         tc.tile_pool(name="tmp", bufs=3) as tpool:

        CALL = bpool.tile([P, N_G * HP], f16)
        SRCB = bpool.tile([P, N_G * HP], f16)
        ACC = bpool.tile([P, N_G * HP], f16)
        # pad strips (+inf) for CALL rows [0:PADH) and [PADH+H:HP),
        # for SRCB rows [0:PADH-1) and [PADH-1+H:HP)
        cstrips = CALL[:].rearrange("p (g h) -> p g h", g=N_G)
        nc.gpsimd.memset(cstrips[:, :, 0:PADH], 2047.0)
        nc.gpsimd.memset(cstrips[:, :, PADH + H:HP], 2047.0)
        strips = SRCB[:].rearrange("p (g h) -> p g h", g=N_G)
        nc.gpsimd.memset(strips[:, :, 0:PADH - 1], 2047.0)
        nc.gpsimd.memset(strips[:, :, PADH - 1 + H:HP], 2047.0)

        call3 = CALL[:].rearrange("p (g h) -> p g h", g=N_G)
        srcb3 = SRCB[:].rearrange("p (g h) -> p g h", g=N_G)
        acc3 = ACC[:].rearrange("p (g h) -> p g h", g=N_G)
        accint = acc3[:, :, PADH:PADH + H]
        d2sv = d2s_d.rearrange("(g ww) b h -> (ww b) g (h)",
                               g=N_G, ww=WS // N_G)
        # three copies of the input land by DMA: source, 1-column-shifted
        # source (odd-shift alignment helper), and the accumulator init.
        nc.sync.dma_start(call3[:, :, PADH:PADH + H], d2sv)
        nc.scalar.dma_start(srcb3[:, :, PADH - 1:PADH - 1 + H], d2sv)
        nc.sync.dma_start(accint, d2sv)

        order = sorted(range(1, R + 1), key=lambda s: (s % 2, -s))
        assert order[-1] == 1
        for s in order:
            if s % 2 == 0:
                lo = call3[:, :, PADH - s:PADH - s + H]
                hi = call3[:, :, PADH + s:PADH + s + H]
            else:
                lo = srcb3[:, :, PADH - s - 1:PADH - s - 1 + H]
                hi = srcb3[:, :, PADH + s - 1:PADH + s - 1 + H]
            pair = tpool.tile([P, N_G * H], f16, tag="pair")
            pair3 = pair[:].rearrange("p (g h) -> p g h", g=N_G)
            nc.vector.tensor_tensor(pair3, lo, hi, Alu.min)
            tmp = tpool.tile([P, N_G * H], f16, tag="tmp")
            tmp3 = tmp[:].rearrange("p (g h) -> p g h", g=N_G)
            if s == 2:
                nc.vector.tensor_scalar(tmp3, pair3, float(s * s), None,
                                        Alu.add)
            else:
                nc.scalar.activation(tmp3, pair3, Act.Copy, bias=float(s * s))
            if s == 1:
                # split the chain tail so sqrt + store of the first half
                # overlap the second half's min
                hh = N_G // 2
                nc.vector.tensor_tensor(accint[:, 0:hh], tmp3[:, 0:hh],
                                        accint[:, 0:hh], Alu.min)
                nc.vector.tensor_tensor(accint[:, hh:N_G], tmp3[:, hh:N_G],
                                        accint[:, hh:N_G], Alu.min)
            else:
                nc.vector.tensor_tensor(accint, tmp3, accint, Alu.min)

        OUT = bpool.tile([P, N_G * H], f32)
        out3 = OUT[:].rearrange("p (g h) -> p g h", g=N_G)
        outv = out_d.rearrange("(g ww) b h -> (ww b) g (h)",
                               g=N_G, ww=WS // N_G)
        hh = N_G // 2
        for half in range(2):
            sl = slice(hh * half, hh * (half + 1))
            nc.scalar.activation(out3[:, sl], accint[:, sl], Act.Sqrt)
            eng = nc.sync if half == 0 else nc.scalar
            eng.dma_start(outv[:, sl], out3[:, sl])


def _split_multi_waits(nc):
    """Walrus in this toolchain encodes at most ONE sync wait per hardware
    instruction.  Hoist extra waits onto same-engine NoOp carriers inserted
    immediately before the over-subscribed instruction (program order on the
    engine preserves the semantics exactly)."""
    import concourse.mybir as mybir

    n = 0
    for fn in nc.m.functions:
        for blk in fn.blocks:
            insts = blk.instructions
            out = []
            for inst in insts:
                si = inst.sync_info
                if si is not None and len(si.on_wait) > 1:
                    waits = list(si.on_wait)
                    for w in waits[:-1]:
                        nop = mybir.InstNoOp(
                            name=f"waitsplit-{n}", ins=[], outs=[])
                        n += 1
                        nop.engine = inst.engine
                        nop.sync_info = mybir.SyncInfo(
                            on_wait=[w], on_update=[])
                        out.append(nop)
                    inst.sync_info = mybir.SyncInfo(
                        on_wait=[waits[-1]], on_update=list(si.on_update))
                out.append(inst)
            blk.instructions = out
    return n


def _make_tc_class():
    """TileContext whose kernel-tail drain is split into one drain per proc.

    The stock tail emits a single sync-engine Drain waiting on every
    outstanding processor; this walrus build only encodes ONE sync wait per
    instruction, so the aggregated drain fails codegen.  Semantics are
    identical — the waits just land on consecutive Drain instructions.
    """
    import concourse.tile as tile
    from concourse.vector_clock import ScopedClock, VectorClock

    class SplitDrainTileContext(tile.TileContext):
        def _drain_and_barrier(self, tick_clock, wait_clock):
            gvc = tick_clock.global_clock
            for proc in range(len(gvc)):
                t = gvc[proc]
                if t <= 0:
                    continue
                d = self.nc.sync.drain()
                sv = VectorClock([0] * len(gvc))
                sv.require_at_least(proc, t)
                wait_clock.add_sem_waits(d.ins, ScopedClock({None: sv}))
            self.nc.all_engine_barrier()
            assert self.sems is not None
            popped = self.nc._tile_sem_poison_stack.pop()
            assert popped is self._sem_poison
            self.nc.clear_and_free_semaphores(
                list(self.sems.allocated().values()))
            self.nc.all_engine_barrier()

    return SplitDrainTileContext


def _build():
    """Build the fused Bass module (done once per process)."""
    import concourse.bass as bass
    import concourse.mybir as mybir

    f16 = mybir.dt.float16
    f32 = mybir.dt.float32
    TC = _make_tc_class()

    nc3 = bass.Bass("TRN2", debug=False, num_devices=NCORES)
    xs3_d = nc3.dram_tensor("xs3", [B, HE, W], f32,
                            kind="ExternalInput").ap()
    ot3_d = nc3.dram_tensor("ot3", [W, B, HS], f32,
                            kind="ExternalOutput").ap()
    with TC(nc3) as tc:
        _k3_body(tc, ot3_d, xs3_d)
    _split_multi_waits(nc3)
    return (nc3,)


def _build_two_launch():
    """Older two-launch pipeline (kept as reference/fallback)."""
    import concourse.bass as bass
    import concourse.mybir as mybir

    f16 = mybir.dt.float16
    f32 = mybir.dt.float32
    TC = _make_tc_class()

    nc1 = bass.Bass("TRN2", debug=False, num_devices=NCORES)
    xs_d = nc1.dram_tensor("xs", [B, HS, W], f32, kind="ExternalInput").ap()
    d2t_d = nc1.dram_tensor("d2t", [W, B, HS], f16, kind="ExternalOutput").ap()
    with TC(nc1) as tc:
        _k1_body(tc, d2t_d, xs_d)
    _split_multi_waits(nc1)

    nc2 = bass.Bass("TRN2", debug=False, num_devices=NCORES)
    d2s_d = nc2.dram_tensor("d2s", [WS, B, H], f16, kind="ExternalInput").ap()
    out_d = nc2.dram_tensor("ot", [WS, B, H], f32, kind="ExternalOutput").ap()
    with TC(nc2) as tc:
        _k2_body(tc, out_d, d2s_d)
    _split_multi_waits(nc2)

    return nc1, nc2


def _host_exact_edt(x):
    """Exact host fallback: banded numpy EDT with growing radius (f32)."""
    INF = np.float32(1e9)
    r = 2 * R
    while True:
        d0 = np.where(x != 0, INF, np.float32(0.0))
        fwd = np.empty_like(d0)
        st = np.full(d0.shape[:2], INF, np.float32)
        for w in range(W):
            st = np.minimum(st + 1.0, d0[:, :, w]); fwd[:, :, w] = st
        st = np.full(d0.shape[:2], INF, np.float32)
        bwd = np.empty_like(d0)
        for w in range(W - 1, -1, -1):
            st = np.minimum(st + 1.0, d0[:, :, w]); bwd[:, :, w] = st
        d2 = np.minimum(fwd, bwd) ** 2
        for axis in (0, 1):
            src = d2
            acc = src.copy()
            rr = min(r, x.shape[axis] - 1)
            for s in range(1, rr + 1):
                sl_lo = [slice(None)] * 3
                sl_hi = [slice(None)] * 3
                sl_lo[axis] = slice(0, x.shape[axis] - s)
                sl_hi[axis] = slice(s, None)
                np.minimum(acc[tuple(sl_lo)], src[tuple(sl_hi)] + s * s,
                           out=acc[tuple(sl_lo)])
                np.minimum(acc[tuple(sl_hi)], src[tuple(sl_lo)] + s * s,
                           out=acc[tuple(sl_hi)])
            d2 = acc
        out = np.sqrt(d2)
        # exact when every per-axis offset fits in the band; r >= max dim
        # means the bands are complete regardless of the value of out
        if out.max() <= r or r >= max(x.shape):
            return out.astype(np.float32)
        r *= 2


def kernel(x):
    global _BUILT
    x = np.asarray(x)
    assert x.shape == (B, H, W)
    if x.dtype != np.float32:
        x = x.astype(np.float32)

    from concourse.bass_utils import run_bass_kernel_spmd

    if _BUILT is None:
        _BUILT = _build()
    (nc3,) = _BUILT
    LAST_RESULTS.clear()

    core_ids = list(range(NCORES))
    # pad H with foreground (nonzero) halo rows; each core gets its slab
    # plus HB halo rows each side, so all three passes are core-local.
    xp = np.pad(x, ((0, 0), (HB, HB), (0, 0)), constant_values=1.0)
    in3 = [{"xs3": np.ascontiguousarray(xp[:, k * HS:k * HS + HE, :])}
           for k in range(NCORES)]
    r3 = run_bass_kernel_spmd(nc3, in3, core_ids)
    LAST_RESULTS.append(r3)
    outt = np.concatenate([r3.results[k]["ot3"] for k in range(NCORES)],
                          axis=2)

    out = outt.transpose(1, 2, 0)          # (w,b,h) -> (b,h,w)

    # Banding is exact iff the true max distance < R + 1 (per-axis integer
    # offsets of the optimal zero are bounded by floor of the distance, and
    # the banded result upper-bounds the true one).
    if float(np.max(out)) >= R + 1:
        out = _host_exact_edt(x)

    nan_mask = np.isnan(x)
    if nan_mask.any():
        out = np.where(nan_mask, np.float32(np.nan), out)
    return out
